# revision 7
# baseline (speedup 1.0000x reference)
"""Trainium2 Bass kernel for unmasked scaled-dot-product attention.

Problem: q, k, v all [4096, 512] fp32.
  out = softmax(q @ k.T / sqrt(512)) @ v

Strategy (8 NeuronCores, SPMD): shard q by rows (512/core), replicate
k, v. Per t-tile (128 keys) of 32, in key-major layout:
  scoresT[t,s] = kT_tile.T @ qT    (4 f16 matmuls over d-chunks)
  ex = exp(scoresT - 2)            (ScalarE, f16 out; the -2 bias keeps
                                    ex <= 36 so the fp8-e4m3 copy used
                                    by the denominator cannot overflow;
                                    num and den scale identically so the
                                    softmax quotient is unchanged)
  outT[e,s] += v_tile.T @ ex       (4 f16 matmuls, PSUM accumulation)
  ex8[pair]  = e4m3(ex)            (DVE cast, idle engine)
  den[1,s]  += ones8.T @ ex8pair   (1 fp8 DoubleRow matmul per PAIR of
                                    tiles: 256-deep contraction at the
                                    fp8 2x rate -- half the PE cost of
                                    the f16 ones-matmul per tile, and
                                    fp8 error on the denominator is a
                                    ~1e-3 multiplicative wobble)
Host: out_c = (outT_c.f32 / den_c).T

Timing structure (from perfetto traces of the previous version):
  - ~7.2us fixed framework preamble (cross-core barriers) on every
    engine; nothing starts earlier.
  - Input DMays issued back-to-back land on DIFFERENT hardware rings
    and share HBM bandwidth round-robin, so the critical first bytes
    (qT + kT tile 0) used to finish at ~12.2us. Fix: chain the chunks
    of each tensor with a 1-element destination overlap -- the WAW
    dependency makes each chunk wait for the previous one, giving
    strict FIFO streams. Two streams run concurrently (qT+kT on sync,
    v on gpsimd after a corner-copy hop), each at ~180 GB/s, with
    chunks sized so data always arrives ahead of consumption.
  - Tile 0's QK is split into 4 column-blocks so the PE can start on
    the first 128 qT columns at ~8.0us instead of waiting for all of
    qT.
  - Tail: PSUM is evacuated as f16 (half the copy time and DMA bytes;
    adds ~5e-4 relative error, budget is 2e-2), DMAs spread across
    four engines' queues.
"""

import math
import os

import numpy as np

S = 4096      # sequence length (queries == keys)
D = 512       # head dim
N_CORES = 8
SH = S // N_CORES          # query rows per core (512)
P = 128                    # partitions
DC = D // P                # d-chunks (4)
TT = S // P                # t-tiles (32)
ET = D // P                # e-tiles of the output dim (4)
NG = TT // 2               # t-tile pairs for the fp8 denominator (16)
EXP_BIAS = -2.0            # exp(sc - 2): keeps e4m3(ex) finite (max 240)

_cache = {}


def _build():
    import concourse.bacc as bacc
    import concourse.tile as tile
    import concourse.mybir as mybir

    f32 = mybir.dt.float32
    f16 = mybir.dt.float16
    f8e4 = mybir.dt.float8e4
    DR = mybir.MatmulPerfMode.DoubleRow

    nc = bacc.Bacc("TRN2", target_bir_lowering=False, debug=False,
                   num_devices=N_CORES)

    # Host prepares partition-major layouts directly; no rearrange views.
    qT_d = nc.dram_tensor("qT", [P, DC, SH], f16, kind="ExternalInput")
    kT_d = nc.dram_tensor("kT", [P, DC, S], f16, kind="ExternalInput")
    v_d = nc.dram_tensor("v", [P, TT * D], f16, kind="ExternalInput")
    ones8_d = nc.dram_tensor("ones8", [P, 2, P], f8e4, kind="ExternalInput")
    outT_d = nc.dram_tensor("outT", [P, ET, SH], f16, kind="ExternalOutput")
    den_d = nc.dram_tensor("denom", [1, SH], f32, kind="ExternalOutput")

    with tile.TileContext(nc) as tc:
        with (
            tc.tile_pool(name="big", bufs=1) as big,
            tc.tile_pool(name="ep", bufs=6) as ep,
            tc.tile_pool(name="x8", bufs=3) as x8p,
            tc.tile_pool(name="outs", bufs=1) as outs,
            tc.tile_pool(name="ps", bufs=3, space="PSUM") as ps,
            tc.tile_pool(name="po", bufs=1, space="PSUM") as po,
        ):
            qT_sb = big.tile([P, DC, SH], f16, tag="qT")
            kT_sb = big.tile([P, DC, S], f16, tag="kT")
            v_sb = big.tile([P, TT * D], f16, tag="v")
            ones8 = big.tile([P, 2, P], f8e4, tag="ones8")
            wz = big.tile([P, SH], f16, tag="warm")
            bias_sb = big.tile([P, 1], f32, tag="bias")
            nc.gpsimd.memset(bias_sb[:], EXP_BIAS)

            # --- input DMA chains -------------------------------------
            # Chunks of one tensor overlap the previous chunk by one
            # column, so the tile framework emits a WAW semaphore wait:
            # each chunk's transfer starts only after the previous chunk
            # lands => strict FIFO per tensor at full per-stream rate.
            nc.vector.memset(wz[:], 0.0)
            nc.sync.dma_start(ones8[:], ones8_d.ap()[:])
            # qT: 4 column chunks (tile-0 QK consumes them in order).
            qc = [(0, 129), (128, 257), (256, 385), (384, 512)]
            for a, b in qc:
                nc.sync.dma_start(qT_sb[:, :, a:b], qT_d.ap()[:, :, a:b])
            # kT: tile 0, tiles 1-3, then 4-tile chunks.
            kc = [(0, 129), (128, 513)] + [
                (t * P, min((t + 4) * P, S) + (1 if (t + 4) * P < S else 0))
                for t in range(4, TT, 4)
            ]
            for a, b in kc:
                nc.sync.dma_start(kT_sb[:, :, a:b], kT_d.ap()[:, :, a:b])
            # v chain on gpsimd's queues, held back behind the qT head
            # by a corner-copy hop so it doesn't steal head bandwidth.
            nc.vector.tensor_copy(v_sb[0:1, 0:1], qT_sb[0:1, 3:4, 511:512])
            vD = TT * D
            vc = [(0, 2 * D + 1)] + [
                (g * D, min((g + 4) * D, vD) + (1 if (g + 4) * D < vD else 0))
                for g in range(2, TT, 4)
            ]
            for a, b in vc:
                nc.gpsimd.dma_start(v_sb[:, a:b], v_d.ap()[:, a:b])

            # --- PSUM accumulators ------------------------------------
            out_ps = [po.tile([P, SH], f32, tag=f"o{e}", name=f"o{e}")
                      for e in range(ET)]
            den_ps = po.tile([P, SH], f32, tag="den")

            # PE warmup on memset data: keeps the PE busy from ~7.5us
            # (preamble end) until the first qT/kT bytes land (~8.0us),
            # and starts the HAM clock ramp early. Results land in an
            # sc-pool buffer that rotates back into real use.
            NWARM = 2
            sc_w = ps.tile([P, SH], f32, tag="sc", name="warm")
            for w in range(NWARM):
                nc.tensor.matmul(
                    sc_w[:], wz[:, 0:P], wz[:],
                    start=(w == 0), stop=(w == NWARM - 1),
                )

            ex_q = {}
            x8_q = {}

            def emit_qk(ti):
                sc = ps.tile([P, SH], f32, tag="sc", name=f"sc{ti}")
                if ti == 0:
                    # Column-blocked: starts on qT cols 0:128 + kT tile
                    # 0 only (~256KB of head data).
                    for b in range(4):
                        cs, ce = b * P, (b + 1) * P
                        for c in range(DC):
                            nc.tensor.matmul(
                                sc[:, cs:ce],
                                kT_sb[:, c, 0:P],
                                qT_sb[:, c, cs:ce],
                                start=(c == 0),
                                stop=(c == DC - 1),
                            )
                else:
                    for c in range(DC):
                        nc.tensor.matmul(
                            sc[:],
                            kT_sb[:, c, ti * P:(ti + 1) * P],
                            qT_sb[:, c, :],
                            start=(c == 0),
                            stop=(c == DC - 1),
                        )
                ex = ep.tile([P, SH], f16, tag="ex", name=f"ex{ti}")
                nc.scalar.activation(
                    ex[:], sc[:], mybir.ActivationFunctionType.Exp,
                    bias=bias_sb[:],
                )
                ex_q[ti] = ex
                g, slot = ti // 2, ti % 2
                if slot == 0:
                    x8_q[g] = x8p.tile([P, 2, SH], f8e4, tag="x8",
                                       name=f"x8_{g}")
                nc.vector.tensor_copy(x8_q[g][:, slot, :], ex[:])

            def emit_av(ti):
                ex = ex_q.pop(ti)
                for e in range(ET):
                    nc.tensor.matmul(
                        out_ps[e][:],
                        v_sb[:, ti * D + e * P:ti * D + (e + 1) * P],
                        ex[:],
                        start=(ti == 0),
                        stop=(ti == TT - 1),
                    )

            def emit_den(g):
                x8 = x8_q.pop(g)
                nc.tensor.matmul(
                    den_ps[:], ones8[:], x8[:],
                    start=(g == 0), stop=(g == NG - 1),
                    perf_mode=DR,
                )

            LAG = 2
            for ti in range(TT):
                emit_qk(ti)
                if ti >= LAG:
                    emit_av(ti - LAG)
                if ti % 2 == 1 and ti >= LAG + 1:
                    emit_den((ti - LAG - 1) // 2)
            for ti in range(TT - LAG, TT):
                emit_av(ti)
            for g in sorted(x8_q):
                emit_den(g)

            # --- tail: f16 evacuation, DMAs spread over engines -------
            outT_sb = outs.tile([P, ET, SH], f16, tag="outT")
            den_sb = outs.tile([1, SH], f32, tag="den_sb")
            dma_eng = [nc.sync, nc.scalar, nc.gpsimd, nc.sync]
            H2 = SH // 2
            for e in range(ET):
                nc.vector.tensor_copy(
                    outT_sb[:, e, 0:H2], out_ps[e][:, 0:H2])
                nc.scalar.activation(
                    outT_sb[:, e, H2:SH], out_ps[e][:, H2:SH],
                    mybir.ActivationFunctionType.Copy,
                )
                dma_eng[e].dma_start(outT_d.ap()[:, e, :], outT_sb[:, e, :])
            nc.vector.tensor_copy(den_sb[:], den_ps[0:1, :])
            nc.gpsimd.dma_start(den_d.ap()[:], den_sb[:])

    nc.compile()
    return nc


def _get_nc():
    if "nc" not in _cache:
        _cache["nc"] = _build()
    return _cache["nc"]


def kernel(q: np.ndarray, k: np.ndarray, v: np.ndarray) -> np.ndarray:
    from concourse import bass_utils
    import ml_dtypes

    assert q.shape == (S, D) and k.shape == (S, D) and v.shape == (S, D)
    scale = 1.0 / math.sqrt(D)

    qs = (np.asarray(q, dtype=np.float32) * scale).astype(np.float16)
    kh = np.asarray(k, dtype=np.float32).astype(np.float16)
    vh = np.asarray(v, dtype=np.float32).astype(np.float16)

    # kT host layout [p, c, t] = k[t, c*128+p]; shared across cores.
    kT = np.ascontiguousarray(kh.T.reshape(DC, P, S).transpose(1, 0, 2))
    # v host layout [p, t*512+e] = v[t*128+p, e]; shared across cores.
    vmat = np.ascontiguousarray(
        vh.reshape(TT, P, D).transpose(1, 0, 2).reshape(P, TT * D))
    ones8 = np.ones((P, 2, P), dtype=ml_dtypes.float8_e4m3)

    in_maps = []
    for c in range(N_CORES):
        qs_c = qs[c * SH:(c + 1) * SH]          # [SH, D]
        qT_c = np.ascontiguousarray(
            qs_c.T.reshape(DC, P, SH).transpose(1, 0, 2))
        in_maps.append({"qT": qT_c, "kT": kT, "v": vmat, "ones8": ones8})

    nc = _get_nc()
    trace = bool(int(os.environ.get("KERNEL_TRACE", "0")))
    res = bass_utils.run_bass_kernel_spmd(
        nc, in_maps, core_ids=list(range(N_CORES)), trace=trace,
    )
    if trace:
        print(f"HW exec time: {res.exec_time_ns} ns")
        _cache["last_result"] = res

    out = np.empty((S, D), dtype=np.float32)
    for c in range(N_CORES):
        outT = res.results[c]["outT"].astype(np.float32)  # [P, ET, SH]
        den = res.results[c]["denom"][0].astype(np.float32)   # [SH]
        full = outT.transpose(1, 0, 2).reshape(D, SH)     # [D(e), SH(s)]
        out[c * SH:(c + 1) * SH] = (full / den[None, :]).T
    return out
